# revision 90
# baseline (speedup 1.0000x reference)
"""DSGCN block kernel for 8x Trainium2 NeuronCores (Bass/Tile) — v2.

Reference (B=16, T=128, N=64, C=128, O=256, K=3, kt=3):
  s[k,n] = sum_m A[k,n,m]
  h[b,t,n,o]   = sum_c x[b,t,n,c] * W_eff[n][c,o],  conv folded into PE taps
  h_gn         = GroupNorm(8 groups over (32 o-chans x all t)) per (b,n)
  y            = h_gn + x @ W_res.T
  out          = gelu(LayerNorm_o(y))   (exact erf gelu; ln_w=1, ln_b=0)

v2 engine plan (per core, nodes n in [8i, 8i+8)):
  - PE:   x transposes -> psA; conv taps -> psH; residual + ypw^T -> psF;
          tiny fp32 matmuls for GN group fold/broadcast.
  - ACT:  fat psH->h_raw drains (1 per batch), Sqrt for rsqrt finalize,
          per-sample LN-affine+gelu straight out of psF PSUM.
  - DVE:  multi-window bn_stats (GN from SBUF fp16, LN from psF fp32),
          moment folds, bn_aggr, reciprocal_approx_fast.
  - Pool: psA->xt copies, per-(blk,sample) GN affine applies (TSP).
  - DMA:  SWDGE cast loads (fp32->fp16) on gpsimd, fat per-2-node chunks;
          HWDGE fp32 stores per node on SP; gout kept fp32 (no cast).
"""

import numpy as np

import concourse.bass as bass
from concourse import mybir
from concourse.tile import TileContext
from concourse.bass_utils import run_bass_kernel_spmd

B, T, N, C, O, K, KT = 16, 128, 64, 128, 256, 3, 3
EPS = 1e-5
NUM_GROUPS = 8
GSIZE = O // NUM_GROUPS       # 32 channels per group
NCORES = 8
NLOC = N // NCORES            # 8 nodes per core
BATCH = 4                     # samples per batch
NB = B // BATCH               # 4 batches per node
FP16 = mybir.dt.float16
FP32 = mybir.dt.float32
AL = mybir.AluOpType
AF = mybir.ActivationFunctionType

_COMPILED = {}


def _split_excess_waits(nc):
    """This walrus build allows at most ONE semaphore wait per instruction.
    Rewrite multi-wait instructions into single-wait NOPs + the instruction
    keeping one wait."""
    wid = [0]
    for fn in nc.m.functions:
        for blk in fn.blocks:
            insts = list(blk.instructions)
            out = []
            changed = False
            for inst in insts:
                si = inst.sync_info
                waits = list(si.on_wait) if si and si.on_wait else []
                if len(waits) > 1:
                    changed = True
                    for w in waits[:-1]:
                        nop = mybir.InstNoOp(name=f"WSPLIT-{wid[0]}", ins=[], outs=[])
                        wid[0] += 1
                        nop.engine = inst.engine
                        nop.sync_info = mybir.SyncInfo(on_wait=[w], on_update=[])
                        out.append(nop)
                    si.on_wait = [waits[-1]]
                out.append(inst)
            if changed:
                blk.instructions = out


def _host_tables(A, dw, W_pw, W_conv, W_res, gn_w, gn_b):
    s = A.sum(axis=2)                                    # [K, N]
    Wk = np.empty((K, C, O), np.float32)
    for k in range(K):
        Wk[k] = dw[k][:, None] * W_pw[:, k * C:(k + 1) * C].T
    W_eff = np.einsum("kn,kco->nco", s, Wk)              # [N, C, O]
    taps = W_conv[:, 0, :]                               # [O, KT]
    # [N, C, KT, O]: tap j contribution weights
    W_eff_dt = W_eff[:, :, None, :] * taps.T[None, None, :, :]

    wres = np.ascontiguousarray(W_res.T)                 # [C, O]

    ident = np.eye(128, dtype=np.float16)

    # group-fold matrix: partitions (o within 128-block) -> 4 groups, 1/32
    g4 = np.zeros((128, 4), np.float32)
    for g in range(4):
        g4[g * GSIZE:(g + 1) * GSIZE, g] = 1.0 / 32.0
    # broadcast matrix: 4 groups -> 128 partitions, columns scaled by gn_w
    # (alpha = gn_w[o] * rsqrt(var_g)); gn_b must be 0 (asserted below).
    g4t = np.zeros((4, 2, 128), np.float32)              # [:, blk, :]
    for blk in range(2):
        for g in range(4):
            for o in range(GSIZE):
                col = g * GSIZE + o
                g4t[g, blk, col] = gn_w[blk * 128 + col]
    # mean broadcast (unscaled) for beta = -mu*alpha
    g4t_mu = np.zeros((4, 128), np.float32)
    for g in range(4):
        g4t_mu[g, g * GSIZE:(g + 1) * GSIZE] = 1.0

    return {
        "wdt": np.ascontiguousarray(
            W_eff_dt.transpose(1, 0, 2, 3)).astype(np.float16),  # [C, N, KT, O]
        "wres": wres.astype(np.float16),
        "ident": ident,
        "g4": g4,
        "g4t": g4t,                                      # [4, 2, 128] f32
        "g4t_mu": g4t_mu,
    }


def _build_kernel():
    nc = bass.Bass("TRN2")

    x_d = nc.dram_tensor("x", [B, T, NLOC, C], FP32, kind="ExternalInput")
    wdt_d = nc.dram_tensor("wdt", [C, NLOC, KT, O], FP16, kind="ExternalInput")
    wres_d = nc.dram_tensor("wres", [C, O], FP16, kind="ExternalInput")
    ident_d = nc.dram_tensor("ident", [128, 128], FP16, kind="ExternalInput")
    g4_d = nc.dram_tensor("g4", [128, 4], FP32, kind="ExternalInput")
    g4t_d = nc.dram_tensor("g4t", [4, 2, 128], FP32, kind="ExternalInput")
    g4tmu_d = nc.dram_tensor("g4t_mu", [4, 128], FP32, kind="ExternalInput")
    out_d = nc.dram_tensor("out", [B, T, NLOC, O], FP32, kind="ExternalOutput")

    with TileContext(nc) as tc:
        with (
            tc.tile_pool(name="const", bufs=1) as cst,
            tc.tile_pool(name="work", bufs=3) as work,
            tc.tile_pool(name="fin", bufs=3) as fin,
            tc.tile_pool(name="bwork", bufs=6) as bwork,
            tc.tile_pool(name="psA", bufs=1, space="PSUM") as psA,
            tc.tile_pool(name="psH", bufs=2, space="PSUM") as psH,
            tc.tile_pool(name="psF", bufs=5, space="PSUM") as psF,
        ):
            # head-critical loads first: x chunk 0, ident, first weight
            # chunk; the rest follows while node 0 computes
            xs = cst.tile([T, B, NLOC, C], FP16)
            nc.gpsimd.dma_start(
                out=xs[:, :, 0:2, :],
                in_=x_d.ap()[:, :, 0:2, :].transpose([1, 0, 2, 3]),
            )
            ident = cst.tile([128, 128], FP16)
            nc.sync.dma_start(out=ident, in_=ident_d.ap())
            wdt = cst.tile([C, NLOC, KT, O], FP16)
            nc.sync.dma_start(out=wdt[:, 0:2], in_=wdt_d.ap()[:, 0:2])
            for k in range(1, NLOC // 2):
                nsl = slice(2 * k, 2 * k + 2)
                nc.gpsimd.dma_start(
                    out=xs[:, :, nsl, :],
                    in_=x_d.ap()[:, :, nsl, :].transpose([1, 0, 2, 3]),
                )
            nc.sync.dma_start(out=wdt[:, 2:5], in_=wdt_d.ap()[:, 2:5])
            nc.sync.dma_start(out=wdt[:, 5:8], in_=wdt_d.ap()[:, 5:8])
            wres = cst.tile([C, O], FP16)
            nc.sync.dma_start(out=wres, in_=wres_d.ap())
            g4 = cst.tile([128, 4], FP32)
            nc.sync.dma_start(out=g4, in_=g4_d.ap())
            g4t = cst.tile([4, 2, 128], FP32)
            nc.sync.dma_start(out=g4t, in_=g4t_d.ap())
            g4tmu = cst.tile([4, 128], FP32)
            nc.sync.dma_start(out=g4tmu, in_=g4tmu_d.ap())

            def new_ctx(ni):
                xt = work.tile([C, B * T], FP16, tag="xt")
                # pairwise-interleaved: [.., pair, t, even/odd]
                h_raw = work.tile([128, 2, B // 2, T, 2], FP16, tag="h_raw")
                stats = work.tile([128, 2, B // 2, 6], FP32, tag="stats")
                gout = work.tile([T, B, O], FP32, tag="gout")
                return dict(ni=ni, xt=xt, h_raw=h_raw, stats=stats, gout=gout,
                            bstate={})

            def emit_transposes(c):
                # all x transposes for the node up-front: producers for the
                # whole A chain, kept ahead of consumer bursts in the streams
                ni, xt = c["ni"], c["xt"]
                for half in range(2):
                    s0 = half * 2 * BATCH
                    ps_xt = psA.tile([C, 2 * BATCH * T], FP16, tag="ps_xt")
                    for s in range(2 * BATCH):
                        nc.tensor.transpose(
                            ps_xt[:, s * T:(s + 1) * T],
                            xs[:, s0 + s, ni, :],
                            ident,
                        )
                    if (ni + half) % 2 == 0:
                        nc.vector.tensor_copy(
                            xt[:, s0 * T:(s0 + 2 * BATCH) * T], ps_xt
                        )
                    else:
                        nc.scalar.copy(
                            xt[:, s0 * T:(s0 + 2 * BATCH) * T], ps_xt
                        )

            def emit_A_unit(c, bi, blk):
                # conv + drain + stats for one (batch, o-block): one PSUM
                # bank (psH bufs=2 lets the next unit's conv overlap).
                # ps_h itself is stored pairwise-interleaved [pr, t, e] so
                # stats (DVE) and the drain (ACT) read PSUM independently —
                # the drain is off the critical path.
                ni, xt, h_raw, stats = c["ni"], c["xt"], c["h_raw"], c["stats"]
                s0 = bi * BATCH
                p0 = bi * (BATCH // 2)
                ps_h = psH.tile([128, BATCH * T], FP32, tag="ps_h")
                ob = slice(blk * 128, (blk + 1) * 128)
                nc.tensor.matmul(
                    ps_h,
                    lhsT=wdt[:, ni, 1, ob],
                    rhs=xt[:, s0 * T:(s0 + BATCH) * T],
                    start=True, stop=False,
                )
                for s in range(BATCH):
                    c0 = (s0 + s) * T
                    nc.tensor.matmul(
                        ps_h[:, s * T + 1:(s + 1) * T],
                        lhsT=wdt[:, ni, 0, ob],
                        rhs=xt[:, c0:c0 + T - 1],
                        start=False, stop=False,
                    )
                    nc.tensor.matmul(
                        ps_h[:, s * T:(s + 1) * T - 1],
                        lhsT=wdt[:, ni, 2, ob],
                        rhs=xt[:, c0 + 1:c0 + T],
                        start=False, stop=(s == BATCH - 1),
                    )
                # drain, writing pairwise-interleaved
                nc.scalar.copy(
                    h_raw[:, blk, p0:p0 + 2, :, :],
                    ps_h.rearrange("p (pr e t) -> p pr t e", pr=2, e=2),
                )
                # bn_stats for the PREVIOUS unit (its drain long done, so
                # DVE never stalls); this unit's stats queue up behind it.
                emit_pend_stats(c)
                c["pend_stats"] = (blk, p0)

            def emit_pend_stats(c):
                pend = c.pop("pend_stats", None)
                if pend is None:
                    return
                blk, p0 = pend
                stats, h_raw = c["stats"], c["h_raw"]
                # evens = sample 2p, odds = sample 2p+1 -> exact per-sample
                # stats from one [p, 6] instruction.
                for pr in range(p0, p0 + 2):
                    nc.vector.bn_stats(
                        out=stats[:, blk, pr, :],
                        in_=h_raw[:, blk, pr, :, :].rearrange(
                            "p t e -> p (t e)"
                        ),
                    )

            def emit_finalize(c):
                # ======== GroupNorm finalize (per node) ========
                # pair 6-tuple: (128, mu_even, 128*var_even,
                #                128, mu_odd, 128*var_odd); col order
                # (pair, e) == sample order since s = 2*pair + e.
                stats = c["stats"]
                muv = stats[:, :, :, 1::3]     # [128, 2, 8, 2]
                M2v = stats[:, :, :, 2::3]     # [128, 2, 8, 2]
                Qt = fin.tile([128, 2, B // 2, 2], FP32, tag="gn_Qt")
                # Qt = 128*mu^2 + M2   (so E[h^2] = Qt/128)
                nc.vector.tensor_tensor(Qt, muv, muv, AL.mult)
                nc.vector.scalar_tensor_tensor(
                    Qt, Qt, 128.0, M2v, AL.mult, AL.add
                )
                # borrow a psF buffer for the tiny matmul outputs:
                # cols 0:64 = broadcast out, 64:96 = mu fold, 96:128 = Q fold
                ps_tb_full = psF.tile([T, O, 2], FP32, tag="ps_fin")
                ps_tb = ps_tb_full.rearrange("t o e -> t (o e)")[:, 0:128]
                nc.tensor.matmul(
                    ps_tb[0:4, 64:96], lhsT=g4, rhs=muv, start=True, stop=True,
                )
                nc.tensor.matmul(
                    ps_tb[0:4, 96:128], lhsT=g4, rhs=Qt, start=True, stop=True,
                )
                gst = fin.tile([4, 2, 2, B], FP32, tag="gst")  # (g, d, blk, s)
                nc.scalar.copy(
                    gst.rearrange("g d b s -> g (d b s)"),
                    ps_tb[0:4, 64:128],
                )
                gmu = gst[:, 0]                # [4, 2, B] group means
                gE2r = gst[:, 1]               # [4, 2, B] 128*E[h^2]
                m2 = fin.tile([4, 2, B], FP32, tag="gn_m2")
                nc.vector.tensor_tensor(m2, gmu, gmu, AL.mult)
                q = fin.tile([4, 2, B], FP32, tag="gn_q")
                nc.vector.tensor_scalar(
                    q, gE2r, 1.0 / 128.0, EPS, AL.mult, AL.add,
                )
                veps = fin.tile([4, 2, B], FP32, tag="gn_veps")
                nc.vector.tensor_tensor(veps, q, m2, AL.subtract)  # var+eps
                rinv = fin.tile([4, 2, B], FP32, tag="gn_rinv")
                nc.vector.reciprocal(out=rinv, in_=veps)
                rs = fin.tile([4, 2, B], FP32, tag="gn_rs")
                nc.scalar.activation(out=rs, in_=rinv, func=AF.Sqrt)
                # broadcast groups -> 128 partitions (alpha includes gn_w)
                ps_b = ps_tb[:, 0:64].rearrange(
                    "p (b s d) -> p b s d", b=2, s=B, d=2
                )
                for blk in range(2):
                    nc.tensor.matmul(
                        ps_b[:, blk, :, 0],
                        lhsT=g4t[:, blk, :],
                        rhs=rs[:, blk, :],
                        start=True, stop=True,
                    )
                    nc.tensor.matmul(
                        ps_b[:, blk, :, 1],
                        lhsT=g4tmu,
                        rhs=gmu[:, blk, :],
                        start=True, stop=True,
                    )
                ab = fin.tile([128, 2, B, 2], FP32, tag="ab")
                nc.scalar.copy(
                    ab.rearrange("p b s d -> p (b s d)"),
                    ps_tb[:, 0:64],
                )
                c["alpha"] = ab[:, :, :, 0]
                beta = fin.tile([128, 2, B], FP32, tag="beta")
                # beta = -mu_bc * alpha   (gn_b == 0 asserted host-side)
                nc.vector.scalar_tensor_tensor(
                    beta, ab[:, :, :, 1], -1.0, c["alpha"], AL.mult, AL.mult
                )
                c["beta"] = beta

            def emit_B_pair(c, bi, u):
                # GN apply + residual/transpose matmuls + LN stats for one
                # sample-pair: one PSUM bank (psF bufs=4)
                xt, h_raw = c["xt"], c["h_raw"]
                alpha, beta = c["alpha"], c["beta"]
                s0 = bi * BATCH
                if u == 0:
                    ypw_t = bwork.tile([128, 2, BATCH * T], FP16, tag="ypw")
                    lstats_t = bwork.tile([T, 2, 6], FP32, tag="lstats")
                    c["ypw"] = ypw_t
                    c["lstats"] = lstats_t
                    c["pair_ps"] = []
                ypw, lstats = c["ypw"], c["lstats"]
                for e in range(2):
                    s = 2 * u + e
                    sg = s0 + s
                    for blk in range(2):
                        nc.gpsimd.tensor_scalar(
                            ypw[:, blk, s * T:(s + 1) * T],
                            h_raw[:, blk, sg // 2, :, sg % 2],
                            alpha[:, blk, sg:sg + 1],
                            beta[:, blk, sg:sg + 1],
                            AL.mult, AL.add,
                        )
                ps_fin = psF.tile([T, O, 2], FP32, tag="ps_fin")
                c["pair_ps"].append(ps_fin)
                for e in range(2):
                    s = 2 * u + e
                    nc.tensor.matmul(
                        ps_fin[:, :, e],
                        lhsT=xt[:, (s0 + s) * T:(s0 + s + 1) * T],
                        rhs=wres,
                        start=True, stop=False,
                    )
                    for blk in range(2):
                        nc.tensor.matmul(
                            ps_fin[:, blk * 128:blk * 128 + 128, e],
                            lhsT=ypw[:, blk, s * T:(s + 1) * T],
                            rhs=ident,
                            start=False, stop=(blk == 1),
                        )
                # interleaved pair -> per-sample (mu, 256*var) from one
                # [T, 6] bn_stats straight off PSUM.
                nc.vector.bn_stats(
                    out=lstats[:, u, :],
                    in_=ps_fin.rearrange("t o e -> t (o e)"),
                )
                if u == 1:
                    # LN smalls right behind the in-order stats (no stall);
                    # the gelu tail runs a slot later, fully dependency-free
                    lmu = lstats[:, :, 1::3]       # [T, 2, 2]
                    lvr = lstats[:, :, 2::3]       # [T, 2, 2] = 256*var
                    lveps = bwork.tile([T, BATCH], FP32, tag="lveps")
                    nc.vector.tensor_scalar(
                        lveps.rearrange("t (u e) -> t u e", u=2),
                        lvr, 1.0 / 256.0, EPS, AL.mult, AL.add,
                    )
                    lrinv = bwork.tile([T, BATCH], FP32, tag="lrinv")
                    nc.vector.reciprocal(out=lrinv, in_=lveps)
                    linv = bwork.tile([T, BATCH], FP32, tag="linv")
                    nc.scalar.activation(out=linv, in_=lrinv, func=AF.Sqrt)
                    lnb = bwork.tile([T, BATCH], FP32, tag="lnb")
                    nc.vector.scalar_tensor_tensor(
                        lnb.rearrange("t (u e) -> t u e", u=2),
                        lmu, -1.0,
                        linv.rearrange("t (u e) -> t u e", u=2),
                        AL.mult, AL.mult,
                    )
                    c["bstate"][bi] = (c["pair_ps"], linv, lnb)

            def emit_B_tail(c, bi):
                gout = c["gout"]
                pair_ps, linv, lnb = c["bstate"].pop(bi)
                s0 = bi * BATCH
                # gelu + LN affine straight out of PSUM
                for s in range(BATCH):
                    nc.scalar.activation(
                        out=gout[:, s0 + s, :],
                        in_=pair_ps[s // 2][:, :, s % 2],
                        func=AF.Gelu,
                        bias=lnb[:, s:s + 1],
                        scale=linv[:, s:s + 1],
                    )
                # store this batch right away (keeps DMA evenly loaded and
                # shortens the pipeline tail)
                nc.sync.dma_start(
                    out=out_d.ap()[s0:s0 + BATCH, :, c["ni"], :].transpose(
                        [1, 0, 2]),
                    in_=gout[:, s0:s0 + BATCH, :],
                )

            def emit_store(c):
                pass

            # Software-pipelined emission with a 2-node lag: per-engine
            # streams execute in program order, so interleave node k-2's B
            # batches (whose GN affine is long since finalized) with node
            # k's A batches. The first and last phases run two nodes of the
            # same kind together to densify pipeline fill/drain.
            phases = [[("A", 0)], [("A", 1)]]
            for j in range(2, NLOC):
                phases.append([("A", j), ("B", j - 2)])
            phases.append([("B", NLOC - 2)])
            phases.append([("B", NLOC - 1)])

            ctx = {}
            for jobs in phases:
                for kind, j in jobs:
                    if kind == "A":
                        ctx[j] = new_ctx(j)
                        emit_transposes(ctx[j])
                for bi in range(NB):
                    for unit in range(2):
                        for kind, j in jobs:
                            if kind == "A":
                                emit_A_unit(ctx[j], bi, unit)
                            else:
                                emit_B_pair(ctx[j], bi, unit)
                    # gelu tail lags one batch slot (fully ready at issue)
                    if bi >= 1:
                        for kind, j in jobs:
                            if kind == "B":
                                emit_B_tail(ctx[j], bi - 1)
                for kind, j in jobs:
                    if kind == "B":
                        emit_B_tail(ctx[j], NB - 1)
                        emit_store(ctx[j])
                        del ctx[j]
                for kind, j in jobs:
                    if kind == "A":
                        emit_pend_stats(ctx[j])
                        emit_finalize(ctx[j])

    _split_excess_waits(nc)
    return nc


LAST_RESULT = {}


def kernel(x, A, dw_weights, W_pw, W_conv, gn_w, gn_b, ln_w, ln_b, W_res,
           _trace=False):
    x = np.asarray(x, np.float32)
    assert np.allclose(np.asarray(ln_w), 1.0) and np.allclose(np.asarray(ln_b), 0.0)
    assert np.allclose(np.asarray(gn_b), 0.0)
    tabs = _host_tables(
        np.asarray(A, np.float32), np.asarray(dw_weights, np.float32),
        np.asarray(W_pw, np.float32), np.asarray(W_conv, np.float32),
        np.asarray(W_res, np.float32), np.asarray(gn_w, np.float32),
        np.asarray(gn_b, np.float32),
    )
    if "nc" not in _COMPILED:
        _COMPILED["nc"] = _build_kernel()
    nc = _COMPILED["nc"]

    in_maps = []
    for core in range(NCORES):
        nsl = slice(core * NLOC, (core + 1) * NLOC)
        in_maps.append({
            "x": np.ascontiguousarray(x[:, :, nsl, :]),
            "wdt": np.ascontiguousarray(tabs["wdt"][:, nsl]),
            "wres": tabs["wres"],
            "ident": tabs["ident"],
            "g4": tabs["g4"],
            "g4t": tabs["g4t"],
            "g4t_mu": tabs["g4t_mu"],
        })
    kw = {}
    if _trace:
        try:
            import antenv.axon_hooks  # noqa: F401
            kw = dict(trace=True, stitch_traces=False)
        except ImportError:
            pass
    res = run_bass_kernel_spmd(nc, in_maps, core_ids=list(range(NCORES)), **kw)
    LAST_RESULT["res"] = res
    out = np.empty((B, T, N, O), np.float32)
    for core in range(NCORES):
        out[:, :, core * NLOC:(core + 1) * NLOC, :] = res.results[core]["out"]
    return out
